# revision 24
# baseline (speedup 1.0000x reference)
"""BiLSTM-CRF NLL kernel for 8 TRN2 NeuronCores.

Sharding: data-parallel over batch. B=128 split into 8 shards of 16
sentences; each core runs both LSTM directions, the fc projection (fused
per-step), the CRF forward pass (exp-domain, renorm every R=8 steps,
capture-at-length), and the gold-path score for its shard.

Recurrence design (per core, per step t, per direction d):
  gates^T [4H=2048, B=16] live in one PSUM tile ps [128, 256], col = m*16+b,
  gate row order permuted to [i | f | o | g]; g rows of W/b pre-scaled by -2
  so tanh(g) = 1 - 2*sigmoid(-2g) comes out of the same sigmoid op.
  Accumulation per m-window: bias (one-hot rhs matmul, start=True) then
  2 k-tiles of Wih @ emb(x_t) (embT resident in SBUF, host-gathered) then
  4 k-tiles of Whh @ h (stop=True).
  Elementwise: 1 sigmoid (Act, 256w) -> gm1 (DVE) -> t2 (DVE), cf (Pool),
  c (DVE, predicated for bwd), tanh_c (Act), h (DVE, bf16 -> hcur slot).
  fc fused: per-step matmuls accumulate emissions^T [12, 512] per 32-step
  chunk in PSUM; first-finishing direction copies to emisT, other adds.
CRF: alpha'_{t+1} = (E @ alpha'_t) * exp(emis_t + fc_b), E = exp(trans)^T;
  renorm every R=8; alpha history in SBUF; per-sentence capture at len-1.
"""

import os
import numpy as np
import ml_dtypes

import concourse.bass as bass
import concourse.bacc as bacc
import concourse.mybir as mybir
import concourse.tile as tile
from concourse.bass import AP

F32 = mybir.dt.float32
BF16 = mybir.dt.bfloat16
I32 = mybir.dt.int32
U8 = mybir.dt.uint8
MUL = mybir.AluOpType.mult
ADD = mybir.AluOpType.add
SUB = mybir.AluOpType.subtract
X = mybir.AxisListType.X
SIG = mybir.ActivationFunctionType.Sigmoid
TANH = mybir.ActivationFunctionType.Tanh

P = 128
B = 16            # batch per core
H = 512
E = 256
G = 2048          # 4H
K = 12
START, STOP = 10, 11
R = 8             # CRF renorm period
NCORES = 8

T = int(os.environ.get("BASS_LSTM_T", "256"))
SKIP = set(os.environ.get("BASS_SKIP", "").split(","))
NE = T // R
NCK = T // 32     # fc chunks


def fv(t, off, pat):
    """Free-dim view of a contiguous [P, F] tile: keep partition pair, replace
    free dims with `pat` (list of [step, count]) at element offset `off`."""
    base = t[:] if not isinstance(t, AP) else t
    part = list(base.ap[0])
    return AP(base.tensor, base.offset + off, [part] + [list(p) for p in pat])


def build(nc):
    dirs = ("f", "b")
    dt = {}

    def din(name, shape, dtype):
        dt[name] = nc.dram_tensor(name, shape, dtype, kind="ExternalInput")
        return dt[name]

    for d in dirs:
        din(f"wihT_{d}", [E, G], BF16)
        din(f"whhT_{d}", [H, G], BF16)
        din(f"bias16_{d}", [16, P], BF16)
        din(f"h0T_{d}", [P, 64], BF16)
        din(f"c0T_{d}", [P, 64], F32)
        din(f"fcWT_{d}", [H, K], BF16)
    din("xembT", [E, T * B], BF16)
    din("onehot16", [16, 256], BF16)
    din("idmat", [P, P], BF16)
    din("mask_b", [T, P, 64], U8)
    din("transT", [K, K], F32)
    din("trans", [K, K], F32)
    din("fcb", [K], F32)
    din("a0", [K, B], F32)
    din("msel", [K, (T // 2) * B], F32)
    din("maskep", [(T // 2 // R) * B], F32)
    din("idseed", [K, K * B], F32)
    din("maskF", [K, (T // 2) * K * B], U8)
    din("mepF", [(T // 2 // R) * B], F32)
    din("islong", [B], F32)
    din("sel", [K, T * B], F32)
    din("counts", [B, 144], F32)
    din("cntb", [B, K], F32)

    nll_o = nc.dram_tensor("nll", [B], F32, kind="ExternalOutput")
    demis_o = nc.dram_tensor("dbg_emis", [K, T * B], F32, kind="ExternalOutput")
    dlogz_o = nc.dram_tensor("dbg_logz", [B], F32, kind="ExternalOutput")
    dgold_o = nc.dram_tensor("dbg_gold", [B], F32, kind="ExternalOutput")

    scr16 = nc.dram_tensor("scr16", [B], F32)
    scr192 = nc.dram_tensor("scr192", [K, B], F32)
    DBG0 = os.environ.get("BASS_DBG0") == "1"
    if DBG0:
        dsg_o = {d: nc.dram_tensor(f"dbg_sg_{d}", [P, 256], F32,
                                   kind="ExternalOutput") for d in dirs}
        dh_o = {d: nc.dram_tensor(f"dbg_h_{d}", [P, 64], BF16,
                                  kind="ExternalOutput") for d in dirs}
        dc_o = {d: nc.dram_tensor(f"dbg_c_{d}", [P, 64], F32,
                                  kind="ExternalOutput") for d in dirs}
        dps_o = {d: nc.dram_tensor(f"dbg_ps_{d}", [P, 256], F32,
                                   kind="ExternalOutput") for d in dirs}

    with tile.TileContext(nc) as tc:
        with tc.tile_pool(name="persist", bufs=1) as pp:
            whh = {d: pp.tile([P, 4 * 16 * P], BF16, name=f"whh{d}", tag=f"whh{d}") for d in dirs}
            wih = {d: pp.tile([P, 2 * 16 * P], BF16, name=f"wih{d}", tag=f"wih{d}") for d in dirs}
            b16 = {d: pp.tile([16, P], BF16, name=f"b16{d}", tag=f"b16{d}") for d in dirs}
            fcw = {d: pp.tile([P, 4 * K], BF16, name=f"fcw{d}", tag=f"fcw{d}") for d in dirs}
            hcur = {d: pp.tile([P, 2 * 64], BF16, name=f"hcur{d}", tag=f"hcur{d}") for d in dirs}
            cst = {d: pp.tile([P, 64], F32, name=f"cst{d}", tag=f"c{d}") for d in dirs}
            oh16 = pp.tile([16, 256], BF16, tag="oh16")
            idm = pp.tile([P, P], BF16, tag="idm")
            embT = pp.tile([P, 2 * T * B], BF16, tag="embT")
            emisT = pp.tile([K, T * B], F32, tag="emisT")
            T2 = T // 2
            NE2 = T2 // R
            expem = pp.tile([K, T * B], F32, tag="expem")
            transTs = pp.tile([K, K], F32, tag="transTs")
            ET = pp.tile([K, K], F32, tag="ET")
            ones12 = pp.tile([K, K], F32, tag="ones12")
            fcb_p = pp.tile([K, 1], F32, tag="fcb_p")
            Fcap = pp.tile([K, K * B], F32, tag="Fcap")
            mFt = pp.tile([K, T2 * K * B], U8, tag="mFt")
            LhF = pp.tile([1, NE2 * B], F32, tag="LhF")
            Fseed = pp.tile([K, K * B], F32, tag="Fseed")

            for d in dirs:
                for k in range(4):
                    nc.sync.dma_start(
                        whh[d][:, k * 16 * P:(k + 1) * 16 * P],
                        dt[f"whhT_{d}"].ap()[k * P:(k + 1) * P, :])
                    nc.sync.dma_start(
                        fcw[d][:, k * K:(k + 1) * K],
                        dt[f"fcWT_{d}"].ap()[k * P:(k + 1) * P, :])
                for k in range(2):
                    nc.sync.dma_start(
                        wih[d][:, k * 16 * P:(k + 1) * 16 * P],
                        dt[f"wihT_{d}"].ap()[k * P:(k + 1) * P, :])
                nc.sync.dma_start(b16[d][:], dt[f"bias16_{d}"].ap()[:])
                nc.sync.dma_start(hcur[d][:, 0:64], dt[f"h0T_{d}"].ap()[:])
                nc.sync.dma_start(cst[d][:], dt[f"c0T_{d}"].ap()[:])
            nc.sync.dma_start(oh16[:], dt["onehot16"].ap()[:])
            nc.sync.dma_start(idm[:], dt["idmat"].ap()[:])
            nc.sync.dma_start(transTs[:], dt["transT"].ap()[:])
            nc.scalar.activation(ET[:], transTs[:], mybir.ActivationFunctionType.Exp)
            nc.vector.memset(ones12[:], 1.0)
            nc.sync.dma_start(fcb_p[:], AP(dt["fcb"], 0, [[1, K], [1, 1]]))
            nc.sync.dma_start(Fseed[:], dt["idseed"].ap()[:])
            nc.vector.memset(Fcap[:], 0.0)
            nc.vector.memset(LhF[:], 0.0)
            nc.sync.dma_start(mFt[:], dt["maskF"].ap()[:])
            for k in range(2):
                nc.sync.dma_start(embT[:, k * T * B:(k + 1) * T * B],
                                  dt["xembT"].ap()[k * P:(k + 1) * P, :])

            # ---- recurrence + fused fc ----
            with tc.tile_pool(name="rec_sbuf", bufs=3) as rp, \
                 tc.tile_pool(name="rec_psum", bufs=2, space="PSUM") as rpp, \
                 tc.tile_pool(name="fc_psum", bufs=1, space="PSUM") as fpp, \
                 tc.tile_pool(name="F_psum", bufs=1, space="PSUM") as Fpp:

                psf, maskch = {}, None
                pend_fc, emit_fc_ref = [], [None]
                Fcur = [Fseed]
                FB = K * B  # 192

                def emit_F_steps(k):
                    # F_t = diag(expem_t) E F_{t-1} for t in [T2+32k, T2+32k+32)
                    for fs in range(32 * k, 32 * k + 32):
                        ta = T2 + fs
                        psF = Fpp.tile([K, FB], F32, name="psF", tag="psF")
                        nc.tensor.matmul(psF[:], ET[:], Fcur[0][:],
                                         start=True, stop=True)
                        Fn = rp.tile([K, FB], F32, name="Fn", tag="Fn")
                        nc.vector.tensor_tensor(
                            Fn[:], psF[:], fv(expem, ta * B, [[1, B], [0, K]]),
                            op=MUL)
                        nc.vector.copy_predicated(
                            Fcap[:], mFt[:, fs * FB:(fs + 1) * FB], Fn[:])
                        if fs % R == R - 1 and fs < T2 - 1:
                            j = (fs + 1) // R
                            psS = Fpp.tile([K, FB], F32, name="psS", tag="psF")
                            nc.tensor.matmul(psS[:], ones12[:], Fn[:],
                                             start=True, stop=True)
                            SF = rp.tile([K, B], F32, name="SF", tag="SF")
                            nc.vector.tensor_reduce(
                                SF[:], fv(psS, 0, [[K, B], [1, K]]),
                                axis=X, op=ADD)
                            rSF = rp.tile([K, B], F32, name="rSF", tag="rSF")
                            nc.vector.reciprocal(rSF[:], SF[:])
                            Fr = rp.tile([K, FB], F32, name="Fr", tag="Fn")
                            nc.vector.tensor_tensor(
                                Fr[:], Fn[:], fv(rSF, 0, [[1, B], [0, K]]),
                                op=MUL)
                            lnSF = rp.tile([1, B], F32, name="lnSF", tag="lnSF")
                            nc.scalar.activation(lnSF[:], SF[0:1, :],
                                                 mybir.ActivationFunctionType.Ln)
                            nc.vector.tensor_tensor(LhF[:, j * B:(j + 1) * B],
                                                    LhF[:, (j - 1) * B:j * B],
                                                    lnSF[:], op=ADD)
                            Fn = Fr
                        Fcur[0] = Fn
                for t in range(0 if "rec" in SKIP else T):
                    sl_in = (t % 2) * 64
                    sl_out = ((t + 1) % 2) * 64
                    if t % R == 0:
                        maskch = rp.tile([P, R * 64], U8, tag="maskch")
                        nc.sync.dma_start(
                            maskch[:], AP(dt["mask_b"], t * P * 64,
                                          [[64, P], [P * 64, R], [1, 64]]))
                    mk = maskch[:, (t % R) * 64:(t % R) * 64 + 64]

                    # gates: per m-window one contiguous accumulation group
                    # (a start=True poisons the whole 2KB PSUM bank for any
                    # other window's pending accumulation, so no interleave);
                    # window order [g,i,f,o] lets sigma over g,i,f start
                    # after window 11.
                    ps_cur = {}
                    for d in dirs:
                        ps = rpp.tile([P, 256], F32, name=f"ps{d}", tag=f"ps{d}")
                        ps_cur[d] = ps
                        tt = t if d == "f" else T - 1 - t
                        for m in range(16):
                            sl = ps[:, m * B:(m + 1) * B]
                            nc.tensor.matmul(sl, b16[d][:],
                                             oh16[:, m * B:(m + 1) * B],
                                             start=True, stop=False)
                            for k in range(2):
                                nc.tensor.matmul(
                                    sl, wih[d][:, (k * 16 + m) * P:(k * 16 + m + 1) * P],
                                    embT[:, k * T * B + tt * B: k * T * B + tt * B + B],
                                    start=False, stop=False)
                            for k in range(4):
                                nc.tensor.matmul(
                                    sl, whh[d][:, (k * 16 + m) * P:(k * 16 + m + 1) * P],
                                    hcur[d][:, sl_in + k * B: sl_in + (k + 1) * B],
                                    start=False, stop=(k == 3))

                    if emit_fc_ref[0] is not None and pend_fc:
                        emit_fc_ref[0](pend_fc.pop(0))

                    sg = {}
                    ps_dbg = dict(ps_cur)
                    sgo = {}
                    for d in dirs:
                        sg[d] = rp.tile([P, 256], F32, name=f"sg{d}", tag=f"sg{d}")
                        nc.scalar.activation(sg[d][:, 0:192], ps_cur[d][:, 0:192], SIG)
                    for d in dirs:
                        sgo[d] = rp.tile([P, 64], F32, name=f"sgo{d}", tag=f"sgo{d}")
                        nc.scalar.activation(sgo[d][:], ps_cur[d][:, 192:256], SIG)

                    # f-chain first on DVE, then b-chain, masking tail last,
                    # to avoid head-of-line blocking across the two chains.
                    th, cfb, t2 = {}, {}, {}
                    cn = None
                    for d in dirs:
                        # gm1 = 1 - 2*sig = tanh of original g (g rows scaled -2)
                        nc.vector.tensor_scalar(
                            out=sg[d][:, 0:64], in0=sg[d][:, 0:64],
                            scalar1=-2.0, scalar2=1.0, op0=MUL, op1=ADD)
                        t2[d] = rp.tile([P, 64], F32, name=f"t2{d}", tag=f"t2{d}")
                        nc.vector.tensor_tensor(t2[d][:], sg[d][:, 64:128],
                                                sg[d][:, 0:64], op=MUL)
                        cfb[d] = rp.tile([P, 64], F32, name=f"cfb{d}", tag=f"cf{d}")
                        nc.vector.tensor_tensor(cfb[d][:], sg[d][:, 128:192],
                                                cst[d][:], op=MUL)
                        if d == "f":
                            nc.vector.tensor_tensor(cst[d][:], cfb[d][:], t2[d][:], op=ADD)
                        else:
                            cn = rp.tile([P, 64], F32, tag="cn")
                            nc.vector.tensor_tensor(cn[:], cfb[d][:], t2[d][:], op=ADD)
                            nc.vector.copy_predicated(cst[d][:], mk, cn[:])
                        th[d] = rp.tile([P, 64], F32, name=f"th{d}", tag=f"th{d}")
                        nc.scalar.activation(th[d][:], cst[d][:], TANH)
                    hslot = {d: hcur[d][:, sl_out:sl_out + 64] for d in dirs}
                    nc.vector.tensor_copy(hslot["b"], hcur["b"][:, sl_in:sl_in + 64])
                    nc.vector.tensor_tensor(hslot["f"], sgo["f"][:],
                                            th["f"][:], op=MUL)
                    hn = rp.tile([P, 64], BF16, tag="hn")
                    nc.vector.tensor_tensor(hn[:], sgo["b"][:],
                                            th["b"][:], op=MUL)
                    nc.vector.copy_predicated(hslot["b"], mk, hn[:])

                    if DBG0 and t == 0:
                        for d in dirs:
                            psc_ = rp.tile([P, 256], F32, name="psc_", tag="psc_")
                            nc.vector.tensor_copy(psc_[:], ps_dbg[d][:])
                            nc.sync.dma_start(dps_o[d].ap()[:], psc_[:])
                            nc.sync.dma_start(dsg_o[d].ap()[:], sg[d][:])
                            nc.sync.dma_start(dc_o[d].ap()[:], cst[d][:])
                            nc.sync.dma_start(
                                dh_o[d].ap()[:],
                                hcur[d][:, sl_out:sl_out + 64])

                    # fused fc for step t, emitted after step t+1's gate
                    # matmuls so it sits behind them in the PE queue
                    def emit_fc(t):
                        slo = ((t + 1) % 2) * 64
                        for d in dirs:
                            if t % 32 == 0:
                                psf[d] = fpp.tile([K, 512], F32, name=f"psf{d}", tag=f"psf{d}")
                            cc = t % 32 if d == "f" else 31 - (t % 32)
                            for k in range(4):
                                nc.tensor.matmul(
                                    psf[d][:, cc * B:(cc + 1) * B],
                                    fcw[d][:, k * K:(k + 1) * K],
                                    hcur[d][:, slo + k * B: slo + (k + 1) * B],
                                    start=(k == 0), stop=(k == 3))
                        if t % 32 == 31:
                            cf_, cb_ = t // 32, NCK - 1 - t // 32
                            for d, ck in (("f", cf_), ("b", cb_)):
                                esl = emisT[:, ck * 512:(ck + 1) * 512]
                                first = (ck < NCK // 2) == (d == "f")
                                if first:
                                    nc.vector.tensor_copy(esl, psf[d][:])
                                else:
                                    nc.vector.tensor_tensor(esl, esl, psf[d][:], op=ADD)
                            ck = cf_
                            if ck >= NCK // 2 and "crf" not in SKIP:
                                nc.scalar.activation(
                                    expem[:, ck * 512:(ck + 1) * 512],
                                    emisT[:, ck * 512:(ck + 1) * 512],
                                    mybir.ActivationFunctionType.Exp,
                                    bias=fcb_p[:, 0:1])
                                emit_F_steps(ck - NCK // 2)
                    emit_fc_ref[0] = emit_fc
                    pend_fc.append(t)

                if "rec" not in SKIP:
                    while pend_fc:
                        emit_fc_ref[0](pend_fc.pop(0))

            nc.sync.dma_start(demis_o.ap()[:], emisT[:])

            # ---- CRF alpha half (t < T2) + combine with overlapped F half ----
            with tc.tile_pool(name="crf_sbuf", bufs=2) as cp, \
                 tc.tile_pool(name="crf_persist", bufs=1) as cpr, \
                 tc.tile_pool(name="crf_psum", bufs=2, space="PSUM") as cpp:
                Estop = cpr.tile([K, 1], F32, tag="Estop")
                nc.scalar.activation(Estop[:], transTs[:, STOP:STOP + 1],
                                     mybir.ActivationFunctionType.Exp)
                for ck in range(0 if "crf" in SKIP else NCK // 2):
                    nc.scalar.activation(
                        expem[:, ck * 512:(ck + 1) * 512],
                        emisT[:, ck * 512:(ck + 1) * 512],
                        mybir.ActivationFunctionType.Exp, bias=fcb_p[:, 0:1])
                a0 = cpr.tile([K, B], F32, tag="a0")
                nc.sync.dma_start(a0[:], dt["a0"].ap()[:])
                hist = cpr.tile([K, T2 * B], F32, tag="hist")
                Lh = cpr.tile([1, (NE2 + 1) * B], F32, tag="Lh")
                nc.vector.memset(Lh[:], 0.0)

                rhs = a0
                rhs_sl = (0, B)
                for t in range(0 if "crf" in SKIP else T2):
                    psc = cpp.tile([K, B], F32, tag="psc")
                    nc.tensor.matmul(psc[:], ET[:],
                                     rhs[:, rhs_sl[0]:rhs_sl[1]],
                                     start=True, stop=True)
                    nc.vector.tensor_tensor(hist[:, t * B:(t + 1) * B], psc[:],
                                            expem[:, t * B:(t + 1) * B], op=MUL)
                    rhs, rhs_sl = hist, (t * B, (t + 1) * B)
                    if t % R == R - 1:
                        j = (t + 1) // R
                        pss = cpp.tile([K, B], F32, tag="pss", bufs=1)
                        nc.tensor.matmul(pss[:], ones12[:], hist[:, t * B:(t + 1) * B],
                                         start=True, stop=True)
                        Ssb = cp.tile([K, B], F32, tag="Ssb")
                        nc.vector.tensor_copy(Ssb[:], pss[:])
                        rS = cp.tile([K, B], F32, tag="rS")
                        nc.vector.reciprocal(rS[:], Ssb[:])
                        rn = cp.tile([K, B], F32, tag="rn")
                        nc.vector.tensor_tensor(rn[:], hist[:, t * B:(t + 1) * B],
                                                rS[:], op=MUL)
                        lnS = cp.tile([1, B], F32, tag="lnS")
                        nc.scalar.activation(lnS[:], Ssb[0:1, :],
                                             mybir.ActivationFunctionType.Ln)
                        nc.vector.tensor_tensor(Lh[:, j * B:(j + 1) * B],
                                                Lh[:, (j - 1) * B:j * B], lnS[:], op=ADD)
                        rhs, rhs_sl = rn, (0, B)

                # short-sentence capture at t = len-1 (< T2)
                mselb = cpr.tile([K, T2 * B], F32, tag="mselb")
                nc.sync.dma_start(mselb[:], dt["msel"].ap()[:])
                nc.vector.tensor_tensor(hist[:], hist[:], mselb[:], op=MUL)
                aend = cp.tile([K, B], F32, tag="aend")
                nc.vector.tensor_reduce(aend[:], fv(hist, 0, [[1, B], [B, T2]]),
                                        axis=X, op=ADD)
                mep = cp.tile([1, NE2 * B], F32, tag="mep")
                nc.sync.dma_start(mep[:], AP(dt["maskep"], 0, [[1, 1], [1, NE2 * B]]))
                prod5 = cp.tile([1, NE2 * B], F32, tag="prod5")
                nc.vector.tensor_tensor(prod5[:], Lh[:, 0:NE2 * B], mep[:], op=MUL)
                Lend = cp.tile([1, B], F32, tag="Lend")
                nc.vector.tensor_reduce(Lend[:], fv(prod5, 0, [[1, B], [B, NE2]]),
                                        axis=X, op=ADD)
                azs = cp.tile([K, B], F32, tag="azs")
                nc.vector.tensor_scalar(out=azs[:], in0=aend[:], scalar1=Estop[:, 0:1],
                                        scalar2=None, op0=MUL)
                ps2 = cpp.tile([K, B], F32, tag="ps2", bufs=1)
                nc.tensor.matmul(ps2[:], ones12[:], azs[:], start=True, stop=True)
                logz0 = cp.tile([1, B], F32, tag="logz0")
                nc.scalar.activation(logz0[:], ps2[0:1, :],
                                     mybir.ActivationFunctionType.Ln)
                logz_s = cp.tile([1, B], F32, tag="logz_s")
                nc.vector.tensor_tensor(logz_s[:], logz0[:], Lend[:], op=ADD)

                # long-sentence path: alpha_end = Fcap @ alpha_T2
                # rhs holds renormalized alpha after t=T2-1 (Lh epoch NE2)
                nc.sync.dma_start(scr192.ap()[:], rhs[:, rhs_sl[0]:rhs_sl[1]])
                a127r = cp.tile([1, K * B], F32, tag="a127r")
                nc.sync.dma_start(a127r[:],
                                  AP(scr192, 0, [[1, 1], [1, B], [B, K]]))
                a127b = cp.tile([K, K * B], F32, tag="a127b")
                nc.gpsimd.partition_broadcast(a127b[:], a127r[:])
                prodL = cp.tile([K, K * B], F32, tag="prodL")
                nc.vector.tensor_tensor(prodL[:], Fcap[:], a127b[:], op=MUL)
                aendL = cp.tile([K, B], F32, tag="aendL")
                nc.vector.tensor_reduce(aendL[:], fv(prodL, 0, [[K, B], [1, K]]),
                                        axis=X, op=ADD)
                azsL = cp.tile([K, B], F32, tag="azsL")
                nc.vector.tensor_scalar(out=azsL[:], in0=aendL[:], scalar1=Estop[:, 0:1],
                                        scalar2=None, op0=MUL)
                ps2L = cpp.tile([K, B], F32, tag="ps2L", bufs=1)
                nc.tensor.matmul(ps2L[:], ones12[:], azsL[:], start=True, stop=True)
                logz0L = cp.tile([1, B], F32, tag="logz0L")
                nc.scalar.activation(logz0L[:], ps2L[0:1, :],
                                     mybir.ActivationFunctionType.Ln)
                mepF = cp.tile([1, NE2 * B], F32, tag="mepF")
                nc.sync.dma_start(mepF[:], AP(dt["mepF"], 0, [[1, 1], [1, NE2 * B]]))
                prodF5 = cp.tile([1, NE2 * B], F32, tag="prodF5")
                nc.vector.tensor_tensor(prodF5[:], LhF[:], mepF[:], op=MUL)
                LendF = cp.tile([1, B], F32, tag="LendF")
                nc.vector.tensor_reduce(LendF[:], fv(prodF5, 0, [[1, B], [B, NE2]]),
                                        axis=X, op=ADD)
                logz_l = cp.tile([1, B], F32, tag="logz_l")
                nc.vector.tensor_tensor(logz_l[:], logz0L[:], LendF[:], op=ADD)
                nc.vector.tensor_tensor(logz_l[:], logz_l[:],
                                        Lh[:, NE2 * B:(NE2 + 1) * B], op=ADD)

                # select per sentence
                islong = cp.tile([1, B], F32, tag="islong")
                nc.sync.dma_start(islong[:], AP(dt["islong"], 0, [[1, 1], [1, B]]))
                logzf = cp.tile([1, B], F32, tag="logzf")
                nc.vector.tensor_tensor(logzf[:], logz_l[:], logz_s[:], op=SUB)
                nc.vector.tensor_tensor(logzf[:], logzf[:], islong[:], op=MUL)
                nc.vector.tensor_tensor(logzf[:], logzf[:], logz_s[:], op=ADD)
                nc.sync.dma_start(AP(dlogz_o, 0, [[1, 1], [1, B]]), logzf[:])

                # ---- gold score ----
                tfl = cp.tile([1, 144], F32, tag="tfl")
                nc.sync.dma_start(tfl[:], AP(dt["trans"], 0, [[1, 1], [1, 144]]))
                tfb = cp.tile([B, 144], F32, tag="tfb")
                nc.gpsimd.partition_broadcast(tfb[:], tfl[:])
                cnts = cp.tile([B, 144], F32, tag="cnts")
                nc.sync.dma_start(cnts[:], dt["counts"].ap()[:])
                pr1 = cp.tile([B, 144], F32, tag="pr1")
                nc.vector.tensor_tensor(pr1[:], cnts[:], tfb[:], op=MUL)
                g1 = cp.tile([B, 1], F32, tag="g1")
                nc.vector.tensor_reduce(g1[:], pr1[:], axis=X, op=ADD)
                fcbr = cp.tile([1, K], F32, tag="fcbr")
                nc.sync.dma_start(fcbr[:], AP(dt["fcb"], 0, [[1, 1], [1, K]]))
                fcbb = cp.tile([B, K], F32, tag="fcbb")
                nc.gpsimd.partition_broadcast(fcbb[:], fcbr[:])
                cntbs = cp.tile([B, K], F32, tag="cntbs")
                nc.sync.dma_start(cntbs[:], dt["cntb"].ap()[:])
                pr2 = cp.tile([B, K], F32, tag="pr2")
                nc.vector.tensor_tensor(pr2[:], cntbs[:], fcbb[:], op=MUL)
                g2 = cp.tile([B, 1], F32, tag="g2")
                nc.vector.tensor_reduce(g2[:], pr2[:], axis=X, op=ADD)
                g12 = cp.tile([B, 1], F32, tag="g12")
                nc.vector.tensor_tensor(g12[:], g1[:], g2[:], op=ADD)
                nc.sync.dma_start(AP(scr16, 0, [[1, B], [1, 1]]), g12[:])
                g12r = cp.tile([1, B], F32, tag="g12r")
                nc.sync.dma_start(g12r[:], AP(scr16, 0, [[1, 1], [1, B]]))

                selb = cpr.tile([K, T * B], F32, tag="selb")
                nc.sync.dma_start(selb[:], dt["sel"].ap()[:])
                nc.vector.tensor_tensor(selb[:], emisT[:], selb[:], op=MUL)
                g3 = cp.tile([K, B], F32, tag="g3")
                nc.vector.tensor_reduce(g3[:], fv(selb, 0, [[1, B], [B, T]]),
                                        axis=X, op=ADD)
                ps3 = cpp.tile([K, B], F32, tag="ps3", bufs=1)
                nc.tensor.matmul(ps3[:], ones12[:], g3[:], start=True, stop=True)
                goldT = cp.tile([1, B], F32, tag="goldT")
                nc.vector.tensor_tensor(goldT[:], g12r[:], ps3[0:1, :], op=ADD)
                nc.sync.dma_start(AP(dgold_o, 0, [[1, 1], [1, B]]), goldT[:])
                nllT = cp.tile([1, B], F32, tag="nllT")
                nc.vector.tensor_tensor(nllT[:], logzf[:], goldT[:], op=SUB)
                nc.sync.dma_start(AP(nll_o, 0, [[1, 1], [1, B]]), nllT[:])
    return nc


_CACHE = {}


def get_program():
    if "nc" not in _CACHE:
        nc = bacc.Bacc("TRN2", target_bir_lowering=False, debug=False,
                       num_devices=NCORES)
        build(nc)
        nc.compile()
        _CACHE["nc"] = nc
    return _CACHE["nc"]


def perm_ifog(w):
    # [4H, ...] rows i,f,g,o -> g,i,f,o (g first so its sigmoid unblocks early)
    return np.concatenate([w[1024:1536], w[0:512], w[512:1024], w[1536:2048]], 0)


def host_prep(inputs):
    f32 = np.float32
    bf = ml_dtypes.bfloat16
    x = np.asarray(inputs["x"]).astype(np.int64)
    lengths = np.asarray(inputs["lengths"]).astype(np.int64)
    tags = np.asarray(inputs["tags"]).astype(np.int64)
    emb = np.asarray(inputs["embedding"], f32)
    trans = np.asarray(inputs["trans"], f32)
    fcW = np.asarray(inputs["fc_W"], f32)
    fcb = np.asarray(inputs["fc_b"], f32)
    h0 = np.asarray(inputs["h0"], f32)
    c0 = np.asarray(inputs["c0"], f32)

    Wd, Bd = {}, {}
    for d in ("f", "b"):
        wih = perm_ifog(np.asarray(inputs[f"W_ih_{d}"], f32)).copy()
        whh = perm_ifog(np.asarray(inputs[f"W_hh_{d}"], f32)).copy()
        bi = perm_ifog(np.asarray(inputs[f"b_ih_{d}"], f32)[:, None])[:, 0]
        bh = perm_ifog(np.asarray(inputs[f"b_hh_{d}"], f32)[:, None])[:, 0]
        bsum = (bi + bh).copy()
        # scale g rows by -2: tanh(g) = 1 - 2*sigmoid(-2g)
        wih[0:512] *= -2.0
        whh[0:512] *= -2.0
        bsum[0:512] *= -2.0
        Wd[d] = (wih.T.astype(bf).copy(), whh.T.astype(bf).copy())
        Bd[d] = bsum.reshape(16, P).astype(bf).copy()

    fcWT = {"f": fcW[:, :H].T.astype(bf).copy(), "b": fcW[:, H:].T.astype(bf).copy()}
    idmat = np.eye(P, dtype=f32).astype(bf)
    oh16 = np.zeros((16, 256), f32)
    for r in range(16):
        oh16[r, r * B:(r + 1) * B] = 1.0
    oh16 = oh16.astype(bf)

    maps = []
    for c in range(NCORES):
        bs = slice(c * B, (c + 1) * B)
        xs = x[bs]            # [16, T]
        ln = lengths[bs]      # [16]
        tg = tags[bs]         # [16, T]
        m = {"trans": trans, "transT": trans.T.astype(f32).copy(), "fcb": fcb,
             "onehot16": oh16, "idmat": idmat}
        # host embedding gather (pure indexing): embT[p, k*T*B + t*16 + b]
        xe = emb[xs]                                  # [16, T, E]
        m["xembT"] = np.ascontiguousarray(
            xe.transpose(2, 1, 0).reshape(2, P, T * B)
        ).reshape(E, T * B).astype(bf)
        for d in ("f", "b"):
            m[f"wihT_{d}"], m[f"whhT_{d}"] = Wd[d]
            m[f"bias16_{d}"] = Bd[d]
            m[f"fcWT_{d}"] = fcWT[d]
            di = 0 if d == "f" else 1
            h0T = h0[di, bs].T.reshape(4, P, B).transpose(1, 0, 2).reshape(P, 64)
            c0T = c0[di, bs].T.reshape(4, P, B).transpose(1, 0, 2).reshape(P, 64)
            m[f"h0T_{d}"] = h0T.astype(bf).copy()
            m[f"c0T_{d}"] = c0T.astype(f32).copy()
        # bwd mask: step s processes tau = T-1-s; valid iff tau < len
        tau = (T - 1 - np.arange(T))[:, None]          # [T, 1]
        mk = (tau < ln[None, :]).astype(f32)           # [T, 16]
        m["mask_b"] = np.broadcast_to(
            mk[:, None, None, :], (T, P, 4, B)).reshape(T, P, 64).astype(np.uint8).copy()
        a0 = np.zeros((K, B), f32); a0[START, :] = 1.0
        m["a0"] = a0
        T2 = T // 2
        NE2 = T2 // R
        lm1 = ln - 1
        # short-sentence capture (dummy slot T2-1 for long sentences)
        cap_s = np.minimum(lm1, T2 - 1)
        msel = np.zeros((K, T2, B), f32)
        msel[:, cap_s, np.arange(B)] = 1.0
        m["msel"] = msel.reshape(K, T2 * B)
        mep = np.zeros((NE2, B), f32)
        mep[np.minimum(cap_s // R, NE2 - 1), np.arange(B)] = 1.0
        m["maskep"] = mep.reshape(-1)
        # F-half: capture at fs = len-1-T2 (dummy fs=0 for short sentences)
        fs_cap = np.where(lm1 >= T2, lm1 - T2, 0)
        mF = np.zeros((T2, B, K), np.uint8)
        mF[fs_cap, np.arange(B), :] = 1
        m["maskF"] = np.broadcast_to(
            mF.reshape(1, T2 * B * K), (K, T2 * B * K)).copy()
        mepF = np.zeros((NE2, B), f32)
        mepF[np.minimum(fs_cap // R, NE2 - 1), np.arange(B)] = 1.0
        m["mepF"] = mepF.reshape(-1)
        m["islong"] = (lm1 >= T2).astype(f32)
        idseed = np.zeros((K, B, K), f32)
        idseed[np.arange(K)[:, None], :, np.arange(K)[:, None]] = 1.0
        m["idseed"] = idseed.reshape(K, B * K)
        tarange = np.arange(T)[None, :]
        valid = tarange < ln[:, None]                  # [16, T]
        selm = np.zeros((K, T, B), f32)
        jj = np.arange(K)[:, None, None]
        selm[:] = (tg.T[None] == jj) & valid.T[None]
        m["sel"] = np.ascontiguousarray(selm.reshape(K, T * B))
        counts = np.zeros((B, 144), f32)
        cntb = np.zeros((B, K), f32)
        for b in range(B):
            L = int(ln[b])
            prev = START
            for t in range(L):
                nx = int(tg[b, t])
                counts[b, nx * K + prev] += 1
                cntb[b, nx] += 1
                prev = nx
            counts[b, STOP * K + prev] += 1
        m["counts"] = counts
        m["cntb"] = cntb
        maps.append(m)
    return maps


def kernel(**inputs):
    from concourse.bass_utils import run_bass_kernel_spmd
    nc = get_program()
    maps = host_prep(inputs)
    res = run_bass_kernel_spmd(nc, maps, core_ids=list(range(NCORES)))
    out = np.concatenate([r["nll"] for r in res.results]).astype(np.float32)
    kernel.last_results = res
    return out


# revision 37
# speedup vs baseline: 3.6368x; 3.6368x over previous
"""BiLSTM-CRF NLL kernel for 8 TRN2 NeuronCores.

Sharding: data-parallel over batch. B=128 split into 8 shards of 16
sentences; each core runs both LSTM directions, the fc projection (fused
per-step), the CRF forward pass (exp-domain, renorm every R=8 steps,
capture-at-length), and the gold-path score for its shard.

Recurrence design (per core, per step t, per direction d):
  gates^T [4H=2048, B=16] live in one PSUM tile ps [128, 256], col = m*16+b,
  gate row order permuted to [i | f | o | g]; g rows of W/b pre-scaled by -2
  so tanh(g) = 1 - 2*sigmoid(-2g) comes out of the same sigmoid op.
  Accumulation per m-window: bias (one-hot rhs matmul, start=True) then
  2 k-tiles of Wih @ emb(x_t) (embT resident in SBUF, host-gathered) then
  4 k-tiles of Whh @ h (stop=True).
  Elementwise: 1 sigmoid (Act, 256w) -> gm1 (DVE) -> t2 (DVE), cf (Pool),
  c (DVE, predicated for bwd), tanh_c (Act), h (DVE, bf16 -> hcur slot).
  fc fused: per-step matmuls accumulate emissions^T [12, 512] per 32-step
  chunk in PSUM; first-finishing direction copies to emisT, other adds.
CRF: alpha'_{t+1} = (E @ alpha'_t) * exp(emis_t + fc_b), E = exp(trans)^T;
  renorm every R=8; alpha history in SBUF; per-sentence capture at len-1.
"""

import os
import numpy as np
import ml_dtypes

import concourse.bass as bass
import concourse.bacc as bacc
import concourse.mybir as mybir
import concourse.tile as tile
from concourse.bass import AP

F32 = mybir.dt.float32
BF16 = mybir.dt.bfloat16
I32 = mybir.dt.int32
U8 = mybir.dt.uint8
MUL = mybir.AluOpType.mult
ADD = mybir.AluOpType.add
SUB = mybir.AluOpType.subtract
X = mybir.AxisListType.X
SIG = mybir.ActivationFunctionType.Sigmoid
TANH = mybir.ActivationFunctionType.Tanh

P = 128
B = 16            # batch per core
H = 512
E = 256
G = 2048          # 4H
K = 12
START, STOP = 10, 11
R = 8             # CRF renorm period
NCORES = 8

T = int(os.environ.get("BASS_LSTM_T", "256"))
SKIP = set(os.environ.get("BASS_SKIP", "").split(","))
NE = T // R
NCK = T // 32     # fc chunks


def fv(t, off, pat):
    """Free-dim view of a contiguous [P, F] tile: keep partition pair, replace
    free dims with `pat` (list of [step, count]) at element offset `off`."""
    base = t[:] if not isinstance(t, AP) else t
    part = list(base.ap[0])
    return AP(base.tensor, base.offset + off, [part] + [list(p) for p in pat])


def build(nc):
    dirs = ("f", "b")
    dt = {}

    def din(name, shape, dtype):
        dt[name] = nc.dram_tensor(name, shape, dtype, kind="ExternalInput")
        return dt[name]

    for d in dirs:
        din(f"wihT_{d}", [E, G], BF16)
        din(f"whhT_{d}", [H, G], BF16)
        din(f"bias16_{d}", [16, P], BF16)
        din(f"h0T_{d}", [P, 64], BF16)
        din(f"c0T_{d}", [P, 64], F32)
        din(f"fcWT_{d}", [H, K], BF16)
    din("xembT", [E, T * B], BF16)
    din("onehot16", [16, 256], BF16)
    din("idmat", [P, P], BF16)
    din("mask_b", [T, P, 64], U8)
    din("transT", [K, K], F32)
    din("trans", [K, K], F32)
    din("fcb", [K], F32)
    din("a0", [K, B], F32)
    din("msel", [K, (T // 2 + 32) * B], F32)
    din("maskep", [(T // 2 // R + 4) * B], F32)
    din("idseed", [K, K * B], BF16)
    if T > 64:
        din("maskF", [K, (T // 2 - 32) * K * B], BF16)
    din("mepF", [(T // 2 // R - 4 + 1) * B], F32)
    din("islong", [B], F32)
    din("sel", [K, T * B], F32)
    din("counts", [B, 144], F32)
    din("cntb", [B, K], F32)

    nll_o = nc.dram_tensor("nll", [B], F32, kind="ExternalOutput")
    demis_o = nc.dram_tensor("dbg_emis", [K, T * B], F32, kind="ExternalOutput")
    dlogz_o = nc.dram_tensor("dbg_logz", [B], F32, kind="ExternalOutput")
    dgold_o = nc.dram_tensor("dbg_gold", [B], F32, kind="ExternalOutput")

    scr16 = nc.dram_tensor("scr16", [B], F32)
    scr192 = nc.dram_tensor("scr192", [K, B], F32)
    DBG0 = os.environ.get("BASS_DBG0") == "1"
    if DBG0:
        dsg_o = {d: nc.dram_tensor(f"dbg_sg_{d}", [P, 256], F32,
                                   kind="ExternalOutput") for d in dirs}
        dh_o = {d: nc.dram_tensor(f"dbg_h_{d}", [P, 64], BF16,
                                  kind="ExternalOutput") for d in dirs}
        dc_o = {d: nc.dram_tensor(f"dbg_c_{d}", [P, 64], F32,
                                  kind="ExternalOutput") for d in dirs}
        dps_o = {d: nc.dram_tensor(f"dbg_ps_{d}", [P, 256], F32,
                                   kind="ExternalOutput") for d in dirs}

    with tile.TileContext(nc) as tc:
        with tc.tile_pool(name="persist", bufs=1) as pp:
            whh = {d: pp.tile([P, 4 * 16 * P], BF16, name=f"whh{d}", tag=f"whh{d}") for d in dirs}
            wih = {d: pp.tile([P, 2 * 16 * P], BF16, name=f"wih{d}", tag=f"wih{d}") for d in dirs}
            b16 = {d: pp.tile([16, P], BF16, name=f"b16{d}", tag=f"b16{d}") for d in dirs}
            fcw = {d: pp.tile([P, 4 * K], BF16, name=f"fcw{d}", tag=f"fcw{d}") for d in dirs}
            hcur = {d: pp.tile([P, 2 * 64], BF16, name=f"hcur{d}", tag=f"hcur{d}") for d in dirs}
            cst = {d: pp.tile([P, 64], F32, name=f"cst{d}", tag=f"c{d}") for d in dirs}
            oh16 = pp.tile([16, 256], BF16, tag="oh16")
            idm = pp.tile([P, P], BF16, tag="idm")
            embT = pp.tile([P, 2 * T * B], BF16, tag="embT")
            emisT = pp.tile([K, T * B], F32, tag="emisT")
            T2 = T // 2
            NE2 = T2 // R
            TF = T2 - 32          # time steps covered by the F product
            NEF = TF // R
            expem = pp.tile([K, T * B], F32, tag="expem")
            transTs = pp.tile([K, K], F32, tag="transTs")
            ET = pp.tile([K, K], F32, tag="ET")
            ones12 = pp.tile([K, K], F32, tag="ones12")
            fcb_p = pp.tile([K, 1], F32, tag="fcb_p")
            Fcap = pp.tile([K, K * B], F32, tag="Fcap")
            ETb = pp.tile([K, K], BF16, tag="ETb")
            if TF > 0:
                mFt = pp.tile([K, TF * K * B], BF16, tag="mFt")
            LhF = pp.tile([1, (NEF + 1) * B], F32, tag="LhF")
            Fseed = pp.tile([K, K * B], BF16, tag="Fseed")
            Ffin = pp.tile([K, K * B], BF16, tag="Ffin")

            for d in dirs:
                for k in range(4):
                    nc.sync.dma_start(
                        whh[d][:, k * 16 * P:(k + 1) * 16 * P],
                        dt[f"whhT_{d}"].ap()[k * P:(k + 1) * P, :])
                    nc.sync.dma_start(
                        fcw[d][:, k * K:(k + 1) * K],
                        dt[f"fcWT_{d}"].ap()[k * P:(k + 1) * P, :])
                for k in range(2):
                    nc.sync.dma_start(
                        wih[d][:, k * 16 * P:(k + 1) * 16 * P],
                        dt[f"wihT_{d}"].ap()[k * P:(k + 1) * P, :])
                nc.sync.dma_start(b16[d][:], dt[f"bias16_{d}"].ap()[:])
                nc.sync.dma_start(hcur[d][:, 0:64], dt[f"h0T_{d}"].ap()[:])
                nc.sync.dma_start(cst[d][:], dt[f"c0T_{d}"].ap()[:])
            nc.sync.dma_start(oh16[:], dt["onehot16"].ap()[:])
            nc.sync.dma_start(idm[:], dt["idmat"].ap()[:])
            nc.sync.dma_start(transTs[:], dt["transT"].ap()[:])
            nc.scalar.activation(ET[:], transTs[:], mybir.ActivationFunctionType.Exp)
            nc.vector.tensor_copy(ETb[:], ET[:])
            nc.vector.memset(ones12[:], 1.0)
            ones12b = pp.tile([K, K], BF16, tag="ones12b")
            nc.vector.memset(ones12b[:], 1.0)
            nc.sync.dma_start(fcb_p[:], AP(dt["fcb"], 0, [[1, K], [1, 1]]))
            nc.sync.dma_start(Fseed[:], dt["idseed"].ap()[:])
            nc.vector.memset(Fcap[:], 0.0)
            nc.vector.memset(LhF[:], 0.0)
            if TF > 0:
                nc.sync.dma_start(mFt[:], dt["maskF"].ap()[:])
            for k in range(2):
                nc.sync.dma_start(embT[:, k * T * B:(k + 1) * T * B],
                                  dt["xembT"].ap()[k * P:(k + 1) * P, :])

            # ---- recurrence + fused fc ----
            with tc.tile_pool(name="rec_sbuf", bufs=3) as rp, \
                 tc.tile_pool(name="rec_psum", bufs=2, space="PSUM") as rpp, \
                 tc.tile_pool(name="rec_psum3", bufs=3, space="PSUM") as rpp3, \
                 tc.tile_pool(name="fc_psum", bufs=1, space="PSUM") as fpp, \
                 tc.tile_pool(name="F_psum", bufs=1, space="PSUM") as Fpp:

                psf, maskch = {}, None
                pend_fc, emit_fc_ref = [], [None]
                Fcur = [Fseed]
                FB = K * B  # 192

                def emit_F_steps(k):
                    # F_t = diag(expem_t) E F_{t-1} for t in [T2+32k, T2+32k+32)
                    for fs in range(32 * k, 32 * k + 32):
                        ta = T2 + fs
                        psF = Fpp.tile([K, FB], F32, name="psF", tag="psF")
                        nc.tensor.matmul(psF[:], ETb[:], Fcur[0][:],
                                         start=True, stop=True)
                        Fn = rp.tile([K, FB], BF16, name="Fn", tag="Fn")
                        nc.vector.tensor_tensor(
                            Fn[:], psF[:], fv(expem, ta * B, [[1, B], [0, K]]),
                            op=MUL)
                        # capture on Pool: Fcap += mask_fs * F_fs
                        Fm = rp.tile([K, FB], BF16, name="Fm", tag="Fm")
                        nc.gpsimd.tensor_tensor(
                            Fm[:], Fn[:], mFt[:, fs * FB:(fs + 1) * FB], op=MUL)
                        nc.gpsimd.tensor_tensor(Fcap[:], Fcap[:], Fm[:], op=ADD)
                        if fs % R == R - 1:
                            j = (fs + 1) // R
                            psS = Fpp.tile([K, FB], F32, name="psS", tag="psF")
                            nc.tensor.matmul(psS[:], ones12b[:], Fn[:],
                                             start=True, stop=True)
                            SF = rp.tile([K, B], F32, name="SF", tag="SF")
                            nc.vector.tensor_reduce(
                                SF[:], fv(psS, 0, [[K, B], [1, K]]),
                                axis=X, op=ADD)
                            rSF = rp.tile([K, B], F32, name="rSF", tag="rSF")
                            nc.vector.reciprocal(rSF[:], SF[:])
                            Fr = rp.tile([K, FB], BF16, name="Fr", tag="Fn")
                            nc.vector.tensor_tensor(
                                Fr[:], Fn[:], fv(rSF, 0, [[1, B], [0, K]]),
                                op=MUL)
                            lnSF = rp.tile([1, B], F32, name="lnSF", tag="lnSF")
                            nc.scalar.activation(lnSF[:], SF[0:1, :],
                                                 mybir.ActivationFunctionType.Ln)
                            nc.vector.tensor_tensor(LhF[:, j * B:(j + 1) * B],
                                                    LhF[:, (j - 1) * B:j * B],
                                                    lnSF[:], op=ADD)
                            Fn = Fr
                        Fcur[0] = Fn
                        if fs == TF - 1:
                            nc.vector.tensor_copy(Ffin[:], Fn[:])
                for t in range(0 if "rec" in SKIP else T):
                    sl_in = (t % 2) * 64
                    sl_out = ((t + 1) % 2) * 64
                    if t % R == 0:
                        maskch = rp.tile([P, R * 64], U8, tag="maskch")
                        nc.sync.dma_start(
                            maskch[:], AP(dt["mask_b"], t * P * 64,
                                          [[64, P], [P * 64, R], [1, 64]]))
                    mk = maskch[:, (t % R) * 64:(t % R) * 64 + 64]

                    # gates: per m-window one contiguous accumulation group
                    # (a start=True poisons the whole 2KB PSUM bank for any
                    # other window's pending accumulation, so no interleave);
                    # window order [g,i,f,o] lets sigma over g,i,f start
                    # after window 11.  Both dirs share one full-bank tile.
                    ps_cur = {}
                    for di, d in enumerate(dirs):
                        pool_d = rpp3 if d == "f" else rpp
                        ps_cur[d] = pool_d.tile([P, 256], F32, name=f"ps{d}", tag=f"ps{d}")
                        tt = t if d == "f" else T - 1 - t
                        for m in range(16):
                            sl = ps_cur[d][:, m * B:(m + 1) * B]
                            nc.tensor.matmul(sl, b16[d][:],
                                             oh16[:, m * B:(m + 1) * B],
                                             start=True, stop=False)
                            for k in range(2):
                                nc.tensor.matmul(
                                    sl, wih[d][:, (k * 16 + m) * P:(k * 16 + m + 1) * P],
                                    embT[:, k * T * B + tt * B: k * T * B + tt * B + B],
                                    start=False, stop=False)
                            for k in range(4):
                                nc.tensor.matmul(
                                    sl, whh[d][:, (k * 16 + m) * P:(k * 16 + m + 1) * P],
                                    hcur[d][:, sl_in + k * B: sl_in + (k + 1) * B],
                                    start=False, stop=(k == 3))

                    if emit_fc_ref[0] is not None and pend_fc:
                        emit_fc_ref[0](pend_fc.pop(0))

                    sg = {}
                    ps_dbg = dict(ps_cur)
                    sgo = {}
                    for di_, d in enumerate(dirs):
                        sg[d] = rp.tile([P, 256], F32, name=f"sg{d}", tag=f"sg{d}")
                        nc.scalar.activation(sg[d][:, 0:192], ps_cur[d][:, 0:192], SIG)
                    for di_, d in enumerate(dirs):
                        sgo[d] = rp.tile([P, 64], F32, name=f"sgo{d}", tag=f"sgo{d}")
                        nc.scalar.activation(sgo[d][:], ps_cur[d][:, 192:256], SIG)

                    # f-chain first on DVE, then b-chain, masking tail last,
                    # to avoid head-of-line blocking across the two chains.
                    th, cfb, t2 = {}, {}, {}
                    cn = None
                    for d in dirs:
                        # gm1 = 1 - 2*sig = tanh of original g (g rows scaled -2)
                        nc.vector.tensor_scalar(
                            out=sg[d][:, 0:64], in0=sg[d][:, 0:64],
                            scalar1=-2.0, scalar2=1.0, op0=MUL, op1=ADD)
                        t2[d] = rp.tile([P, 64], F32, name=f"t2{d}", tag=f"t2{d}")
                        nc.vector.tensor_tensor(t2[d][:], sg[d][:, 64:128],
                                                sg[d][:, 0:64], op=MUL)
                        cfb[d] = rp.tile([P, 64], F32, name=f"cfb{d}", tag=f"cf{d}")
                        nc.vector.tensor_tensor(cfb[d][:], sg[d][:, 128:192],
                                                cst[d][:], op=MUL)
                        if d == "f":
                            nc.vector.tensor_tensor(cst[d][:], cfb[d][:], t2[d][:], op=ADD)
                        else:
                            cn = rp.tile([P, 64], F32, tag="cn")
                            nc.vector.tensor_tensor(cn[:], cfb[d][:], t2[d][:], op=ADD)
                            nc.vector.copy_predicated(cst[d][:], mk, cn[:])
                        th[d] = rp.tile([P, 64], F32, name=f"th{d}", tag=f"th{d}")
                        nc.scalar.activation(th[d][:], cst[d][:], TANH)
                    hslot = {d: hcur[d][:, sl_out:sl_out + 64] for d in dirs}
                    nc.vector.tensor_copy(hslot["b"], hcur["b"][:, sl_in:sl_in + 64])
                    nc.vector.tensor_tensor(hslot["f"], sgo["f"][:],
                                            th["f"][:], op=MUL)
                    hn = rp.tile([P, 64], BF16, tag="hn")
                    nc.vector.tensor_tensor(hn[:], sgo["b"][:],
                                            th["b"][:], op=MUL)
                    nc.vector.copy_predicated(hslot["b"], mk, hn[:])

                    if DBG0 and t == 0:
                        for d in dirs:
                            psc_ = rp.tile([P, 256], F32, name="psc_", tag="psc_")
                            nc.vector.tensor_copy(psc_[:], ps_dbg[d][:])
                            nc.sync.dma_start(dps_o[d].ap()[:], psc_[:])
                            nc.sync.dma_start(dsg_o[d].ap()[:], sg[d][:])
                            nc.sync.dma_start(dc_o[d].ap()[:], cst[d][:])
                            nc.sync.dma_start(
                                dh_o[d].ap()[:],
                                hcur[d][:, sl_out:sl_out + 64])

                    # fused fc for step t, emitted after step t+1's gate
                    # matmuls so it sits behind them in the PE queue
                    def emit_fc(t):
                        slo = ((t + 1) % 2) * 64
                        for d in dirs:
                            if t % 32 == 0:
                                psf[d] = fpp.tile([K, 512], F32, name=f"psf{d}", tag=f"psf{d}")
                            cc = t % 32 if d == "f" else 31 - (t % 32)
                            for k in range(4):
                                nc.tensor.matmul(
                                    psf[d][:, cc * B:(cc + 1) * B],
                                    fcw[d][:, k * K:(k + 1) * K],
                                    hcur[d][:, slo + k * B: slo + (k + 1) * B],
                                    start=(k == 0), stop=(k == 3))
                        if t % 32 == 31:
                            cf_, cb_ = t // 32, NCK - 1 - t // 32
                            for d, ck in (("f", cf_), ("b", cb_)):
                                esl = emisT[:, ck * 512:(ck + 1) * 512]
                                first = (ck < NCK // 2) == (d == "f")
                                if first:
                                    nc.vector.tensor_copy(esl, psf[d][:])
                                else:
                                    nc.vector.tensor_tensor(esl, esl, psf[d][:], op=ADD)
                            ck = cf_
                            if ck >= NCK // 2 and "crf" not in SKIP:
                                nc.scalar.activation(
                                    expem[:, ck * 512:(ck + 1) * 512],
                                    emisT[:, ck * 512:(ck + 1) * 512],
                                    mybir.ActivationFunctionType.Exp,
                                    bias=fcb_p[:, 0:1])
                                if ck < NCK - 1:
                                    emit_F_steps(ck - NCK // 2)
                    emit_fc_ref[0] = emit_fc
                    pend_fc.append(t)

                if "rec" not in SKIP:
                    while pend_fc:
                        emit_fc_ref[0](pend_fc.pop(0))

            nc.sync.dma_start(demis_o.ap()[:], emisT[:])

            # ---- CRF alpha half (t < T2) + combine with overlapped F half ----
            with tc.tile_pool(name="crf_sbuf", bufs=2) as cp, \
                 tc.tile_pool(name="crf_persist", bufs=1) as cpr, \
                 tc.tile_pool(name="crf_psum", bufs=2, space="PSUM") as cpp:
                Estop = cpr.tile([K, 1], F32, tag="Estop")
                nc.scalar.activation(Estop[:], transTs[:, STOP:STOP + 1],
                                     mybir.ActivationFunctionType.Exp)
                for ck in range(0 if "crf" in SKIP else NCK // 2):
                    nc.scalar.activation(
                        expem[:, ck * 512:(ck + 1) * 512],
                        emisT[:, ck * 512:(ck + 1) * 512],
                        mybir.ActivationFunctionType.Exp, bias=fcb_p[:, 0:1])
                a0 = cpr.tile([K, B], F32, tag="a0")
                nc.sync.dma_start(a0[:], dt["a0"].ap()[:])
                hist = cpr.tile([K, (T2 + 32) * B], F32, tag="hist")
                Lh = cpr.tile([1, (NE2 + 4) * B], F32, tag="Lh")
                nc.vector.memset(Lh[:], 0.0)

                rhs = a0
                rhs_sl = (0, B)
                for t in range(0 if "crf" in SKIP else T2):
                    psc = cpp.tile([K, B], F32, tag="psc")
                    nc.tensor.matmul(psc[:], ET[:],
                                     rhs[:, rhs_sl[0]:rhs_sl[1]],
                                     start=True, stop=True)
                    nc.vector.tensor_tensor(hist[:, t * B:(t + 1) * B], psc[:],
                                            expem[:, t * B:(t + 1) * B], op=MUL)
                    rhs, rhs_sl = hist, (t * B, (t + 1) * B)
                    if t % R == R - 1:
                        j = (t + 1) // R
                        pss = cpp.tile([K, B], F32, tag="pss", bufs=1)
                        nc.tensor.matmul(pss[:], ones12[:], hist[:, t * B:(t + 1) * B],
                                         start=True, stop=True)
                        Ssb = cp.tile([K, B], F32, tag="Ssb")
                        nc.vector.tensor_copy(Ssb[:], pss[:])
                        rS = cp.tile([K, B], F32, tag="rS")
                        nc.vector.reciprocal(rS[:], Ssb[:])
                        rn = cp.tile([K, B], F32, tag="rn")
                        nc.vector.tensor_tensor(rn[:], hist[:, t * B:(t + 1) * B],
                                                rS[:], op=MUL)
                        lnS = cp.tile([1, B], F32, tag="lnS")
                        nc.scalar.activation(lnS[:], Ssb[0:1, :],
                                             mybir.ActivationFunctionType.Ln)
                        nc.vector.tensor_tensor(Lh[:, j * B:(j + 1) * B],
                                                Lh[:, (j - 1) * B:j * B], lnS[:], op=ADD)
                        rhs, rhs_sl = rn, (0, B)

                # combine: A_start = Fcur @ alpha_T2 (Fcur = identity when TF=0)
                nc.sync.dma_start(scr192.ap()[:], rhs[:, rhs_sl[0]:rhs_sl[1]])
                a127r = cp.tile([1, K * B], F32, tag="a127r")
                nc.sync.dma_start(a127r[:],
                                  AP(scr192, 0, [[1, 1], [1, B], [B, K]]))
                a127b = cp.tile([K, K * B], F32, tag="a127b")
                nc.gpsimd.partition_broadcast(a127b[:], a127r[:])
                prodS = cp.tile([K, K * B], F32, tag="prodS")
                Flast = Ffin if TF > 0 and "crf" not in SKIP and "rec" not in SKIP else Fseed
                nc.vector.tensor_tensor(prodS[:], Flast[:], a127b[:], op=MUL)
                astart = cpr.tile([K, B], F32, tag="astart")
                nc.vector.tensor_reduce(astart[:], fv(prodS, 0, [[K, B], [1, K]]),
                                        axis=X, op=ADD)
                # keep LA (alpha-only corrections) for the mid path, then
                # fold F's log corrections into the running Lh chain
                LA = cpr.tile([1, B], F32, tag="LA")
                nc.vector.tensor_copy(LA[:], Lh[:, NE2 * B:(NE2 + 1) * B])
                nc.vector.tensor_tensor(Lh[:, NE2 * B:(NE2 + 1) * B],
                                        Lh[:, NE2 * B:(NE2 + 1) * B],
                                        LhF[:, NEF * B:(NEF + 1) * B], op=ADD)

                # sequential tail over the last 32 time steps
                rhs, rhs_sl = astart, (0, B)
                for tt in range(0 if "crf" in SKIP else 32):
                    ta = T - 32 + tt
                    hp = T2 + tt
                    psc = cpp.tile([K, B], F32, tag="psc")
                    nc.tensor.matmul(psc[:], ET[:],
                                     rhs[:, rhs_sl[0]:rhs_sl[1]],
                                     start=True, stop=True)
                    nc.vector.tensor_tensor(hist[:, hp * B:(hp + 1) * B], psc[:],
                                            expem[:, ta * B:(ta + 1) * B], op=MUL)
                    rhs, rhs_sl = hist, (hp * B, (hp + 1) * B)
                    if tt % R == R - 1 and tt < 31:
                        j = NE2 + 1 + tt // R
                        pss = cpp.tile([K, B], F32, tag="pss", bufs=1)
                        nc.tensor.matmul(pss[:], ones12[:],
                                         hist[:, hp * B:(hp + 1) * B],
                                         start=True, stop=True)
                        Ssb = cp.tile([K, B], F32, tag="Ssb")
                        nc.vector.tensor_copy(Ssb[:], pss[:])
                        rS = cp.tile([K, B], F32, tag="rS")
                        nc.vector.reciprocal(rS[:], Ssb[:])
                        rn = cp.tile([K, B], F32, tag="rn")
                        nc.vector.tensor_tensor(rn[:], hist[:, hp * B:(hp + 1) * B],
                                                rS[:], op=MUL)
                        lnS = cp.tile([1, B], F32, tag="lnS")
                        nc.scalar.activation(lnS[:], Ssb[0:1, :],
                                             mybir.ActivationFunctionType.Ln)
                        nc.vector.tensor_tensor(Lh[:, j * B:(j + 1) * B],
                                                Lh[:, (j - 1) * B:j * B], lnS[:], op=ADD)
                        rhs, rhs_sl = rn, (0, B)

                # capture at t = len-1 for short (t < T2) and tail (t >= T-32)
                mselb = cpr.tile([K, (T2 + 32) * B], F32, tag="mselb")
                nc.sync.dma_start(mselb[:], dt["msel"].ap()[:])
                nc.vector.tensor_tensor(hist[:], hist[:], mselb[:], op=MUL)
                aend = cp.tile([K, B], F32, tag="aend")
                nc.vector.tensor_reduce(aend[:], fv(hist, 0, [[1, B], [B, T2 + 32]]),
                                        axis=X, op=ADD)
                mep = cp.tile([1, (NE2 + 4) * B], F32, tag="mep")
                nc.sync.dma_start(mep[:], AP(dt["maskep"], 0,
                                             [[1, 1], [1, (NE2 + 4) * B]]))
                prod5 = cp.tile([1, (NE2 + 4) * B], F32, tag="prod5")
                nc.vector.tensor_tensor(prod5[:], Lh[:, 0:(NE2 + 4) * B], mep[:], op=MUL)
                Lend = cp.tile([1, B], F32, tag="Lend")
                nc.vector.tensor_reduce(Lend[:], fv(prod5, 0, [[1, B], [B, NE2 + 4]]),
                                        axis=X, op=ADD)
                azs = cp.tile([K, B], F32, tag="azs")
                nc.vector.tensor_scalar(out=azs[:], in0=aend[:], scalar1=Estop[:, 0:1],
                                        scalar2=None, op0=MUL)
                ps2 = cpp.tile([K, B], F32, tag="ps2", bufs=1)
                nc.tensor.matmul(ps2[:], ones12[:], azs[:], start=True, stop=True)
                logz0 = cp.tile([1, B], F32, tag="logz0")
                nc.scalar.activation(logz0[:], ps2[0:1, :],
                                     mybir.ActivationFunctionType.Ln)
                logz_s = cp.tile([1, B], F32, tag="logz_s")
                nc.vector.tensor_tensor(logz_s[:], logz0[:], Lend[:], op=ADD)

                # mid-sentence path (T2 <= len-1 < T-32): alpha_end = Fcap @ alpha_T2
                prodL = cp.tile([K, K * B], F32, tag="prodL")
                nc.vector.tensor_tensor(prodL[:], Fcap[:], a127b[:], op=MUL)
                aendL = cp.tile([K, B], F32, tag="aendL")
                nc.vector.tensor_reduce(aendL[:], fv(prodL, 0, [[K, B], [1, K]]),
                                        axis=X, op=ADD)
                azsL = cp.tile([K, B], F32, tag="azsL")
                nc.vector.tensor_scalar(out=azsL[:], in0=aendL[:], scalar1=Estop[:, 0:1],
                                        scalar2=None, op0=MUL)
                nc.vector.tensor_scalar(out=azsL[:], in0=azsL[:], scalar1=1e-30,
                                        scalar2=None, op0=ADD)
                ps2L = cpp.tile([K, B], F32, tag="ps2L", bufs=1)
                nc.tensor.matmul(ps2L[:], ones12[:], azsL[:], start=True, stop=True)
                logz0L = cp.tile([1, B], F32, tag="logz0L")
                nc.scalar.activation(logz0L[:], ps2L[0:1, :],
                                     mybir.ActivationFunctionType.Ln)
                mepF = cp.tile([1, (NEF + 1) * B], F32, tag="mepF")
                nc.sync.dma_start(mepF[:], AP(dt["mepF"], 0,
                                              [[1, 1], [1, (NEF + 1) * B]]))
                prodF5 = cp.tile([1, (NEF + 1) * B], F32, tag="prodF5")
                nc.vector.tensor_tensor(prodF5[:], LhF[:], mepF[:], op=MUL)
                LendF = cp.tile([1, B], F32, tag="LendF")
                nc.vector.tensor_reduce(LendF[:], fv(prodF5, 0, [[1, B], [B, NEF + 1]]),
                                        axis=X, op=ADD)
                logz_l = cp.tile([1, B], F32, tag="logz_l")
                nc.vector.tensor_tensor(logz_l[:], logz0L[:], LendF[:], op=ADD)
                nc.vector.tensor_tensor(logz_l[:], logz_l[:], LA[:], op=ADD)

                # select per sentence (isF = mid-range length)
                islong = cp.tile([1, B], F32, tag="islong")
                nc.sync.dma_start(islong[:], AP(dt["islong"], 0, [[1, 1], [1, B]]))
                logzf = cp.tile([1, B], F32, tag="logzf")
                nc.vector.tensor_tensor(logzf[:], logz_l[:], logz_s[:], op=SUB)
                nc.vector.tensor_tensor(logzf[:], logzf[:], islong[:], op=MUL)
                nc.vector.tensor_tensor(logzf[:], logzf[:], logz_s[:], op=ADD)
                nc.sync.dma_start(AP(dlogz_o, 0, [[1, 1], [1, B]]), logzf[:])

                # ---- gold score ----
                tfl = cp.tile([1, 144], F32, tag="tfl")
                nc.sync.dma_start(tfl[:], AP(dt["trans"], 0, [[1, 1], [1, 144]]))
                tfb = cp.tile([B, 144], F32, tag="tfb")
                nc.gpsimd.partition_broadcast(tfb[:], tfl[:])
                cnts = cp.tile([B, 144], F32, tag="cnts")
                nc.sync.dma_start(cnts[:], dt["counts"].ap()[:])
                pr1 = cp.tile([B, 144], F32, tag="pr1")
                nc.vector.tensor_tensor(pr1[:], cnts[:], tfb[:], op=MUL)
                g1 = cp.tile([B, 1], F32, tag="g1")
                nc.vector.tensor_reduce(g1[:], pr1[:], axis=X, op=ADD)
                fcbr = cp.tile([1, K], F32, tag="fcbr")
                nc.sync.dma_start(fcbr[:], AP(dt["fcb"], 0, [[1, 1], [1, K]]))
                fcbb = cp.tile([B, K], F32, tag="fcbb")
                nc.gpsimd.partition_broadcast(fcbb[:], fcbr[:])
                cntbs = cp.tile([B, K], F32, tag="cntbs")
                nc.sync.dma_start(cntbs[:], dt["cntb"].ap()[:])
                pr2 = cp.tile([B, K], F32, tag="pr2")
                nc.vector.tensor_tensor(pr2[:], cntbs[:], fcbb[:], op=MUL)
                g2 = cp.tile([B, 1], F32, tag="g2")
                nc.vector.tensor_reduce(g2[:], pr2[:], axis=X, op=ADD)
                g12 = cp.tile([B, 1], F32, tag="g12")
                nc.vector.tensor_tensor(g12[:], g1[:], g2[:], op=ADD)
                nc.sync.dma_start(AP(scr16, 0, [[1, B], [1, 1]]), g12[:])
                g12r = cp.tile([1, B], F32, tag="g12r")
                nc.sync.dma_start(g12r[:], AP(scr16, 0, [[1, 1], [1, B]]))

                selb = cpr.tile([K, T * B], F32, tag="selb")
                nc.sync.dma_start(selb[:], dt["sel"].ap()[:])
                nc.vector.tensor_tensor(selb[:], emisT[:], selb[:], op=MUL)
                g3 = cp.tile([K, B], F32, tag="g3")
                nc.vector.tensor_reduce(g3[:], fv(selb, 0, [[1, B], [B, T]]),
                                        axis=X, op=ADD)
                ps3 = cpp.tile([K, B], F32, tag="ps3", bufs=1)
                nc.tensor.matmul(ps3[:], ones12[:], g3[:], start=True, stop=True)
                goldT = cp.tile([1, B], F32, tag="goldT")
                nc.vector.tensor_tensor(goldT[:], g12r[:], ps3[0:1, :], op=ADD)
                nc.sync.dma_start(AP(dgold_o, 0, [[1, 1], [1, B]]), goldT[:])
                nllT = cp.tile([1, B], F32, tag="nllT")
                nc.vector.tensor_tensor(nllT[:], logzf[:], goldT[:], op=SUB)
                nc.sync.dma_start(AP(nll_o, 0, [[1, 1], [1, B]]), nllT[:])
    return nc


_CACHE = {}


def get_program():
    if "nc" not in _CACHE:
        nc = bacc.Bacc("TRN2", target_bir_lowering=False, debug=False,
                       num_devices=NCORES)
        build(nc)
        nc.compile()
        _CACHE["nc"] = nc
    return _CACHE["nc"]


def perm_ifog(w):
    # [4H, ...] rows i,f,g,o -> g,i,f,o (g first so its sigmoid unblocks early)
    return np.concatenate([w[1024:1536], w[0:512], w[512:1024], w[1536:2048]], 0)


def host_prep(inputs):
    f32 = np.float32
    bf = ml_dtypes.bfloat16
    x = np.asarray(inputs["x"]).astype(np.int64)
    lengths = np.asarray(inputs["lengths"]).astype(np.int64)
    tags = np.asarray(inputs["tags"]).astype(np.int64)
    emb = np.asarray(inputs["embedding"], f32)
    trans = np.asarray(inputs["trans"], f32)
    fcW = np.asarray(inputs["fc_W"], f32)
    fcb = np.asarray(inputs["fc_b"], f32)
    h0 = np.asarray(inputs["h0"], f32)
    c0 = np.asarray(inputs["c0"], f32)

    Wd, Bd = {}, {}
    for d in ("f", "b"):
        wih = perm_ifog(np.asarray(inputs[f"W_ih_{d}"], f32)).copy()
        whh = perm_ifog(np.asarray(inputs[f"W_hh_{d}"], f32)).copy()
        bi = perm_ifog(np.asarray(inputs[f"b_ih_{d}"], f32)[:, None])[:, 0]
        bh = perm_ifog(np.asarray(inputs[f"b_hh_{d}"], f32)[:, None])[:, 0]
        bsum = (bi + bh).copy()
        # scale g rows by -2: tanh(g) = 1 - 2*sigmoid(-2g)
        wih[0:512] *= -2.0
        whh[0:512] *= -2.0
        bsum[0:512] *= -2.0
        Wd[d] = (wih.T.astype(bf).copy(), whh.T.astype(bf).copy())
        Bd[d] = bsum.reshape(16, P).astype(bf).copy()

    fcWT = {"f": fcW[:, :H].T.astype(bf).copy(), "b": fcW[:, H:].T.astype(bf).copy()}
    idmat = np.eye(P, dtype=f32).astype(bf)
    oh16 = np.zeros((16, 256), f32)
    for r in range(16):
        oh16[r, r * B:(r + 1) * B] = 1.0
    oh16 = oh16.astype(bf)

    maps = []
    for c in range(NCORES):
        bs = slice(c * B, (c + 1) * B)
        xs = x[bs]            # [16, T]
        ln = lengths[bs]      # [16]
        tg = tags[bs]         # [16, T]
        m = {"trans": trans, "transT": trans.T.astype(f32).copy(), "fcb": fcb,
             "onehot16": oh16, "idmat": idmat}
        # host embedding gather (pure indexing): embT[p, k*T*B + t*16 + b]
        xe = emb[xs]                                  # [16, T, E]
        m["xembT"] = np.ascontiguousarray(
            xe.transpose(2, 1, 0).reshape(2, P, T * B)
        ).reshape(E, T * B).astype(bf)
        for d in ("f", "b"):
            m[f"wihT_{d}"], m[f"whhT_{d}"] = Wd[d]
            m[f"bias16_{d}"] = Bd[d]
            m[f"fcWT_{d}"] = fcWT[d]
            di = 0 if d == "f" else 1
            h0T = h0[di, bs].T.reshape(4, P, B).transpose(1, 0, 2).reshape(P, 64)
            c0T = c0[di, bs].T.reshape(4, P, B).transpose(1, 0, 2).reshape(P, 64)
            m[f"h0T_{d}"] = h0T.astype(bf).copy()
            m[f"c0T_{d}"] = c0T.astype(f32).copy()
        # bwd mask: step s processes tau = T-1-s; valid iff tau < len
        tau = (T - 1 - np.arange(T))[:, None]          # [T, 1]
        mk = (tau < ln[None, :]).astype(f32)           # [T, 16]
        m["mask_b"] = np.broadcast_to(
            mk[:, None, None, :], (T, P, 4, B)).reshape(T, P, 64).astype(np.uint8).copy()
        a0 = np.zeros((K, B), f32); a0[START, :] = 1.0
        m["a0"] = a0
        T2 = T // 2
        NE2 = T2 // R
        TF = T2 - 32
        NEF = TF // R
        lm1 = ln - 1
        is_mid = (lm1 >= T2) & (lm1 < T - 32)
        is_tail = lm1 >= T - 32
        # hist capture position: short at lm1, tail at T2 + (lm1-(T-32)), mid dummy 0
        cap_h = np.where(is_tail, T2 + lm1 - (T - 32), np.where(is_mid, 0, lm1))
        msel = np.zeros((K, T2 + 32, B), f32)
        msel[:, cap_h, np.arange(B)] = 1.0
        m["msel"] = msel.reshape(K, (T2 + 32) * B)
        ep_h = np.where(is_tail, NE2 + (lm1 - (T - 32)) // R,
                        np.where(is_mid, 0, lm1 // R))
        mep = np.zeros((NE2 + 4, B), f32)
        mep[ep_h, np.arange(B)] = 1.0
        m["maskep"] = mep.reshape(-1)
        # F-half capture (mid sentences only)
        fs_cap = np.where(is_mid, lm1 - T2, 0)
        if TF > 0:
            mF = np.zeros((TF, B, K), f32)
            mF[fs_cap, np.arange(B), :] = np.where(is_mid, 1, 0)[None, :, None][0][:, None]
            m["maskF"] = np.broadcast_to(
                mF.reshape(1, TF * B * K), (K, TF * B * K)).copy()
        mepF = np.zeros((NEF + 1, B), f32)
        mepF[np.where(is_mid, fs_cap // R, 0), np.arange(B)] = 1.0
        m["mepF"] = mepF.reshape(-1)
        m["islong"] = is_mid.astype(f32)
        idseed = np.zeros((K, B, K), f32)
        idseed[np.arange(K)[:, None], :, np.arange(K)[:, None]] = 1.0
        m["idseed"] = idseed.reshape(K, B * K).astype(bf)
        tarange = np.arange(T)[None, :]
        valid = tarange < ln[:, None]                  # [16, T]
        selm = np.zeros((K, T, B), f32)
        jj = np.arange(K)[:, None, None]
        selm[:] = (tg.T[None] == jj) & valid.T[None]
        m["sel"] = np.ascontiguousarray(selm.reshape(K, T * B))
        counts = np.zeros((B, 144), f32)
        cntb = np.zeros((B, K), f32)
        for b in range(B):
            L = int(ln[b])
            prev = START
            for t in range(L):
                nx = int(tg[b, t])
                counts[b, nx * K + prev] += 1
                cntb[b, nx] += 1
                prev = nx
            counts[b, STOP * K + prev] += 1
        m["counts"] = counts
        m["cntb"] = cntb
        maps.append(m)
    return maps


def kernel(**inputs):
    from concourse.bass_utils import run_bass_kernel_spmd
    nc = get_program()
    maps = host_prep(inputs)
    res = run_bass_kernel_spmd(nc, maps, core_ids=list(range(NCORES)))
    out = np.concatenate([r["nll"] for r in res.results]).astype(np.float32)
    kernel.last_results = res
    return out
